# revision 18
# baseline (speedup 1.0000x reference)
"""Trainium2 Bass kernel for masked additive-attention pooling.

Reference math (per batch b):
    whhn = encoding @ W_h.T                            # [B, D]
    M    = tanh(X @ W_y.T + whhn[:, None, :])          # [B, T, D]
    a    = sigmoid(M @ w_a)                            # [B, T]
    e    = exp(a); den = sum(e * mask); w = e * mask / den
    out  = sum_t w[t] * X[t]                           # [B, D]

Sharding: data-parallel over batch B=32 across 8 cores (4 batches/core).
Weights replicated. Host does layout transforms only (weight transposes,
column repacks, bf16 casts); all FLOPs run on device.

Device strategy per core:
  - X^T arrives as a host-pretransposed bf16 tensor; one large DMA per
    512-token tile (the on-chip alternatives -- PE transpose-mode or the
    xbar DMA-transpose -- cost PE cycles / hit an xbar corruption bug).
  - z^T[e,tok] = Wy^T . X^T as bf16 matmuls (1 cycle/row, FWL).
  - tanh fused with the per-(e,b) whhn bias on the Scalar engine.
  - logits: a_pre[tok] = th^T . w_a as tiny N=1 matmuls accumulated in
    PSUM -> column layout, directly usable as pooling weights.
  - sigmoid via 0.5 + 0.5*tanh(x/2) folded into ACT scale/bias so the
    whole kernel uses one activation table set (exp_and_others).
  - pooling num/den as fp32r matmuls against the fp32 X tiles.
"""

import sys

if "/opt/trn_rl_repo" not in sys.path:
    sys.path.insert(0, "/opt/trn_rl_repo")

import numpy as np
import ml_dtypes

import concourse.bacc as bacc
import concourse.mybir as mybir
import concourse.tile as tile
from concourse.bass_utils import run_bass_kernel_spmd

F32 = mybir.dt.float32
F32R = mybir.dt.float32r
BF16 = mybir.dt.bfloat16
FP8 = mybir.dt.float8e4
AF = mybir.ActivationFunctionType

N_CORES = 8
B, T, D = 32, 2048, 1024
B_LOC = B // N_CORES          # 4 batches per core
NTOK = B_LOC * T              # 8192 tokens per core
TILE_T = 512                  # tokens per big tile
NBT = NTOK // TILE_T          # 16 big tiles
BT_PER_B = T // TILE_T        # 4 big tiles per batch
CH = TILE_T // 128            # 4 chunks of 128 tokens per big tile
KD = D // 128                 # 8 contraction chunks
EB = D // 128                 # 8 output-feature blocks

_CACHE = {}


def build():
    nc = bacc.Bacc("TRN2", target_bir_lowering=False, debug=False,
                   num_devices=N_CORES)

    x = nc.dram_tensor("x", [NTOK, D], F32R, kind="ExternalInput").ap()
    xt = nc.dram_tensor("xt", [D, NTOK], FP8, kind="ExternalInput").ap()
    wyt = nc.dram_tensor("wyt", [EB, 128, KD * 128], FP8,
                         kind="ExternalInput").ap()
    wht = nc.dram_tensor("wht", [EB, 128, KD * 128], BF16,
                         kind="ExternalInput").ap()
    enc_cols = nc.dram_tensor("enc_cols", [128, KD * B_LOC], BF16,
                              kind="ExternalInput").ap()
    wa_cols = nc.dram_tensor("wa_cols", [128, EB], FP8,
                             kind="ExternalInput").ap()
    mask_cols = nc.dram_tensor("mask_cols", [128, NTOK // 128], F32,
                               kind="ExternalInput").ap()
    ones = nc.dram_tensor("ones", [128, 2], F32R, kind="ExternalInput").ap()
    out = nc.dram_tensor("out", [B_LOC, D], F32, kind="ExternalOutput").ap()

    # [p, k, tok] view of the pretransposed bf16 X^T
    xt3 = xt.rearrange("(k p) n -> p k n", p=128)
    # [j, p, c, d] view of fp32 X
    x4 = x.rearrange("(j c p) d -> j p c d", p=128, c=CH)

    with tile.TileContext(nc) as tc:
        with tc.tile_pool(name="consts", bufs=1) as cp, \
             tc.tile_pool(name="wy", bufs=1) as wyp, \
             tc.tile_pool(name="xnat", bufs=3) as xp, \
             tc.tile_pool(name="xt", bufs=3) as xtp, \
             tc.tile_pool(name="th", bufs=2) as thp, \
             tc.tile_pool(name="small", bufs=3) as smp, \
             tc.tile_pool(name="mps", bufs=1, space="PSUM") as psum:

            state = {}

            def load_xt(j, split=1):
                t = xtp.tile([128, KD * TILE_T], FP8, tag="xt",
                             name=f"xt_{j}")
                kk = KD // split
                for s in range(split):
                    nc.sync.dma_start(
                        t[:, s * kk * TILE_T:(s + 1) * kk * TILE_T]
                        .rearrange("p (k n) -> p k n", k=kk),
                        xt3[:, s * kk:(s + 1) * kk,
                            j * TILE_T:(j + 1) * TILE_T])
                state[("xt", j)] = t

            def load_xnat(j):
                t = xp.tile([128, CH * D], F32R, tag="xn", name=f"x_{j}")
                nc.sync.dma_start(
                    t[:].rearrange("p (c d) -> p c d", c=CH), x4[j])
                state[("xn", j)] = t

            # ---- phase 0: weights + constants ----
            load_xt(0)
            wy_sb = []
            for eb in range(EB):
                t = wyp.tile([128, KD * 128], FP8, tag=f"wy{eb}")
                nc.sync.dma_start(t[:], wyt[eb])
                wy_sb.append(t)
            enc_sb = cp.tile([128, KD * B_LOC], BF16)
            nc.sync.dma_start(enc_sb[:], enc_cols[:])
            wh_sb = []
            whp_cm = tc.tile_pool(name="wh", bufs=1)
            whp = whp_cm.__enter__()
            for eb in range(EB):
                t = whp.tile([128, KD * 128], BF16, tag=f"wh{eb}")
                nc.sync.dma_start(t[:], wht[eb])
                wh_sb.append(t)
            ones_sb = cp.tile([128, 2], F32R)
            half_sb = cp.tile([128, 1], F32)
            nc.vector.memset(half_sb[:], 0.5)
            wa_sb = cp.tile([128, EB], FP8)
            mask_sb = cp.tile([128, NTOK // 128], F32)
            whhn_sb = cp.tile([128, EB * B_LOC], F32)
            nc.sync.dma_start(ones_sb[:], ones[:])
            nc.sync.dma_start(wa_sb[:], wa_cols[:])
            nc.sync.dma_start(mask_sb[:], mask_cols[:])
            load_xnat(0)

            def emit_whhn():
                for eb in range(EB):
                    php = psum.tile([128, B_LOC], F32, tag="apre", bufs=2,
                                    name=f"php_{eb}")
                    for k in range(KD):
                        nc.tensor.matmul(
                            php[:], wh_sb[eb][:, k * 128:(k + 1) * 128],
                            enc_sb[:, k * B_LOC:(k + 1) * B_LOC],
                            start=(k == 0), stop=(k == KD - 1))
                    nc.vector.tensor_copy(
                        whhn_sb[:, eb * B_LOC:(eb + 1) * B_LOC], php[:])

            # ---- main loop ----
            def emit_z_mm(j, eb):
                zp = psum.tile([128, TILE_T], F32, tag="z", bufs=3,
                               name=f"z_{j}_{eb}")
                xt_t = state[("xt", j)]
                for q in range(KD // 2):
                    nc.tensor.matmul(
                        zp[:],
                        wy_sb[eb][:, q * 256:(q + 1) * 256]
                        .rearrange("p (i m) -> p i m", i=2),
                        xt_t[:, 2 * q * TILE_T:(2 * q + 2) * TILE_T]
                        .rearrange("p (i n) -> p i n", i=2),
                        start=(q == 0), stop=(q == KD // 2 - 1),
                        perf_mode=mybir.MatmulPerfMode.DoubleRow)
                return zp

            def emit_tanh(j, eb, zp):
                b = j // BT_PER_B
                th_t = thp.tile([128, TILE_T], FP8, tag=f"th{eb}",
                                name=f"th_{j}_{eb}")
                nc.scalar.activation(
                    th_t[:], zp[:], AF.Tanh, scale=1.0 / 32.0,
                    bias=whhn_sb[:, eb * B_LOC + b:eb * B_LOC + b + 1])
                state[("th", j, eb)] = th_t

            def emit_z(j, ebs):
                for eb in ebs:
                    emit_tanh(j, eb, emit_z_mm(j, eb))

            def emit_apre(j):
                app = psum.tile([128, CH], F32, tag="apre", bufs=2,
                                name=f"apre_{j}")
                for c in range(CH):
                    for eb in range(EB):
                        nc.tensor.matmul(
                            app[:, c:c + 1],
                            state[("th", j, eb)][:, c * 128:(c + 1) * 128],
                            wa_sb[:, eb:eb + 1],
                            start=(eb == 0), stop=(eb == EB - 1))
                for eb in range(EB):
                    state.pop(("th", j, eb))
                # sigmoid(x) = 0.5 + 0.5*tanh(x/2); both tanh and exp live
                # in the exp_and_others table set -> no table reloads.
                tcol = smp.tile([128, CH], F32, tag="tcol", name=f"tcol_{j}")
                nc.scalar.activation(tcol[:], app[:], AF.Tanh,
                     scale=0.5 / 32.0)
                ex = smp.tile([128, CH], F32, tag="ex", name=f"ex_{j}")
                nc.scalar.activation(ex[:], tcol[:], AF.Exp,
                                     bias=half_sb[:], scale=0.5)
                ew = smp.tile([128, CH], F32R, tag="ew", name=f"ew_{j}")
                nc.vector.tensor_mul(
                    ew[:], ex[:], mask_sb[:, j * CH:(j + 1) * CH])
                state[("ew", j)] = ew

            def emit_pool(j):
                b = j // BT_PER_B
                if j % BT_PER_B == 0:
                    state[("num", b)] = [
                        psum.tile([1, 512], F32, tag=f"num{dn}", bufs=1,
                                  name=f"num_{b}_{dn}")
                        for dn in range(2)]
                    state[("den", b)] = psum.tile([1, 2], F32, tag="den",
                                                  bufs=1, name=f"den_{b}")
                ew = state.pop(("ew", j))
                xn = state.pop(("xn", j))
                first = j % BT_PER_B == 0
                last = j % BT_PER_B == BT_PER_B - 1
                for c in range(CH):
                    st = first and c == 0
                    sp = last and c == CH - 1
                    for dn in range(2):
                        nc.tensor.matmul(
                            state[("num", b)][dn][:],
                            ew[:, c:c + 1],
                            xn[:, c * D + dn * 512:c * D + (dn + 1) * 512],
                            start=st, stop=sp)
                    nc.tensor.matmul(
                        state[("den", b)][:], ew[:, c:c + 1], ones_sb[:],
                        start=st, stop=sp)
                if last:
                    rec = smp.tile([1, 1], F32, tag="rec", name=f"rec_{b}")
                    nc.vector.reciprocal(rec[:], state[("den", b)][:, 0:1])
                    ob = smp.tile([1, D], F32, tag="ob", bufs=2,
                                  name=f"ob_{b}")
                    for dn in range(2):
                        nc.vector.tensor_scalar_mul(
                            ob[:, dn * 512:(dn + 1) * 512],
                            state[("num", b)][dn][:], rec[:])
                    nc.sync.dma_start(out[b:b + 1, :], ob[:])
                    state.pop(("num", b))
                    state.pop(("den", b))

            for j in range(NBT):
                if j + 1 < NBT:
                    load_xt(j + 1)
                    load_xnat(j + 1)
                if j == 0:
                    zps = [emit_z_mm(0, eb) for eb in range(3)]
                    emit_whhn()
                    whp_cm.__exit__(None, None, None)
                    for eb in range(3):
                        emit_tanh(0, eb, zps[eb])
                    emit_z(0, range(3, 4))
                else:
                    emit_z(j, range(0, 4))
                if j > 0:
                    emit_apre(j - 1)
                if j > 1:
                    emit_pool(j - 2)
                emit_z(j, range(4, 8))
            emit_apre(NBT - 1)
            emit_pool(NBT - 2)
            emit_pool(NBT - 1)

    nc.compile()
    return nc


def _host_pack(full_input, encoding, mask, W_h, W_y, w_a):
    """Build the per-core input maps (layout transforms / casts only)."""
    wyT = np.ascontiguousarray(W_y.T)  # [d, e]
    whT = np.ascontiguousarray(W_h.T)
    wyt_rows = np.empty((EB, 128, KD * 128), ml_dtypes.float8_e4m3)
    wht_rows = np.empty((EB, 128, KD * 128), ml_dtypes.bfloat16)
    for eb in range(EB):
        for k in range(KD):
            wyt_rows[eb, :, k * 128:(k + 1) * 128] = (
                32.0 * wyT[k * 128:(k + 1) * 128, eb * 128:(eb + 1) * 128])
            wht_rows[eb, :, k * 128:(k + 1) * 128] = \
                whT[k * 128:(k + 1) * 128, eb * 128:(eb + 1) * 128]
    wa_c = np.ascontiguousarray(
        32.0 * w_a.reshape(KD, 128).T).astype(ml_dtypes.float8_e4m3)
    ones = np.ones((128, 2), np.float32)

    in_maps = []
    for i in range(N_CORES):
        sl = slice(i * B_LOC, (i + 1) * B_LOC)
        x_i = np.ascontiguousarray(
            full_input[sl].reshape(NTOK, D).astype(np.float32))
        xt_i = np.ascontiguousarray(x_i.T.astype(ml_dtypes.float8_e4m3))
        enc_i = np.ascontiguousarray(
            encoding[sl].T.reshape(KD, 128, B_LOC).transpose(1, 0, 2)
            .reshape(128, KD * B_LOC)).astype(ml_dtypes.bfloat16)
        mask_i = np.ascontiguousarray(
            mask[sl].reshape(NTOK // 128, 128).T.astype(np.float32))
        in_maps.append({
            "x": x_i, "xt": xt_i, "wyt": wyt_rows, "wht": wht_rows,
            "enc_cols": enc_i, "wa_cols": wa_c, "mask_cols": mask_i,
            "ones": ones,
        })
    return in_maps


def run(inputs, trace=False):
    if "nc" not in _CACHE:
        _CACHE["nc"] = build()
    nc = _CACHE["nc"]
    in_maps = _host_pack(**inputs)
    res = run_bass_kernel_spmd(nc, in_maps, core_ids=list(range(N_CORES)),
                               trace=trace)
    out = np.concatenate([res.results[i]["out"] for i in range(N_CORES)],
                         axis=0)
    return out, res


def kernel(**inputs):
    out, _ = run(inputs, trace=False)
    return out


# revision 19
# speedup vs baseline: 1.0058x; 1.0058x over previous
"""Trainium2 Bass kernel for masked additive-attention pooling.

Reference math (per batch b):
    whhn = encoding @ W_h.T                            # [B, D]
    M    = tanh(X @ W_y.T + whhn[:, None, :])          # [B, T, D]
    a    = sigmoid(M @ w_a)                            # [B, T]
    e    = exp(a); den = sum(e * mask); w = e * mask / den
    out  = sum_t w[t] * X[t]                           # [B, D]

Sharding: data-parallel over batch B=32 across 8 cores (4 batches/core).
Weights replicated. Host does layout transforms only (weight transposes,
column repacks, bf16 casts); all FLOPs run on device.

Device strategy per core:
  - X^T arrives as a host-pretransposed bf16 tensor; one large DMA per
    512-token tile (the on-chip alternatives -- PE transpose-mode or the
    xbar DMA-transpose -- cost PE cycles / hit an xbar corruption bug).
  - z^T[e,tok] = Wy^T . X^T as bf16 matmuls (1 cycle/row, FWL).
  - tanh fused with the per-(e,b) whhn bias on the Scalar engine.
  - logits: a_pre[tok] = th^T . w_a as tiny N=1 matmuls accumulated in
    PSUM -> column layout, directly usable as pooling weights.
  - sigmoid via 0.5 + 0.5*tanh(x/2) folded into ACT scale/bias so the
    whole kernel uses one activation table set (exp_and_others).
  - pooling num/den as fp32r matmuls against the fp32 X tiles.
"""

import sys

if "/opt/trn_rl_repo" not in sys.path:
    sys.path.insert(0, "/opt/trn_rl_repo")

import numpy as np
import ml_dtypes

import concourse.bacc as bacc
import concourse.mybir as mybir
import concourse.tile as tile
from concourse.bass_utils import run_bass_kernel_spmd

F32 = mybir.dt.float32
F32R = mybir.dt.float32r
BF16 = mybir.dt.bfloat16
FP8 = mybir.dt.float8e4
AF = mybir.ActivationFunctionType

N_CORES = 8
B, T, D = 32, 2048, 1024
B_LOC = B // N_CORES          # 4 batches per core
NTOK = B_LOC * T              # 8192 tokens per core
TILE_T = 512                  # tokens per big tile
NBT = NTOK // TILE_T          # 16 big tiles
BT_PER_B = T // TILE_T        # 4 big tiles per batch
CH = TILE_T // 128            # 4 chunks of 128 tokens per big tile
KD = D // 128                 # 8 contraction chunks
EB = D // 128                 # 8 output-feature blocks

_CACHE = {}


def build():
    nc = bacc.Bacc("TRN2", target_bir_lowering=False, debug=False,
                   num_devices=N_CORES)

    x = nc.dram_tensor("x", [NTOK, D], F32R, kind="ExternalInput").ap()
    xt = nc.dram_tensor("xt", [D, NTOK], FP8, kind="ExternalInput").ap()
    wyt = nc.dram_tensor("wyt", [EB, 128, KD * 128], FP8,
                         kind="ExternalInput").ap()
    wht = nc.dram_tensor("wht", [EB, 128, KD * 128], BF16,
                         kind="ExternalInput").ap()
    enc_cols = nc.dram_tensor("enc_cols", [128, KD * B_LOC], BF16,
                              kind="ExternalInput").ap()
    wa_cols = nc.dram_tensor("wa_cols", [128, EB], FP8,
                             kind="ExternalInput").ap()
    mask_cols = nc.dram_tensor("mask_cols", [128, NTOK // 128], F32,
                               kind="ExternalInput").ap()
    ones = nc.dram_tensor("ones", [128, 2], F32R, kind="ExternalInput").ap()
    out = nc.dram_tensor("out", [B_LOC, D], F32, kind="ExternalOutput").ap()

    # [p, k, tok] view of the pretransposed bf16 X^T
    xt3 = xt.rearrange("(k p) n -> p k n", p=128)
    # [j, p, c, d] view of fp32 X
    x4 = x.rearrange("(j c p) d -> j p c d", p=128, c=CH)

    with tile.TileContext(nc) as tc:
        with tc.tile_pool(name="consts", bufs=1) as cp, \
             tc.tile_pool(name="wy", bufs=1) as wyp, \
             tc.tile_pool(name="xnat", bufs=3) as xp, \
             tc.tile_pool(name="xt", bufs=3) as xtp, \
             tc.tile_pool(name="th", bufs=2) as thp, \
             tc.tile_pool(name="small", bufs=3) as smp, \
             tc.tile_pool(name="mps", bufs=1, space="PSUM") as psum:

            state = {}

            def load_xt(j, split=1):
                t = xtp.tile([128, KD * TILE_T], FP8, tag="xt",
                             name=f"xt_{j}")
                kk = KD // split
                for s in range(split):
                    nc.sync.dma_start(
                        t[:, s * kk * TILE_T:(s + 1) * kk * TILE_T]
                        .rearrange("p (k n) -> p k n", k=kk),
                        xt3[:, s * kk:(s + 1) * kk,
                            j * TILE_T:(j + 1) * TILE_T])
                state[("xt", j)] = t

            def load_xnat(j):
                t = xp.tile([128, CH * D], F32R, tag="xn", name=f"x_{j}")
                nc.sync.dma_start(
                    t[:].rearrange("p (c d) -> p c d", c=CH), x4[j])
                state[("xn", j)] = t

            # ---- phase 0: weights + constants ----
            load_xt(0)
            wy_sb = []
            for eb in range(EB):
                t = wyp.tile([128, KD * 128], FP8, tag=f"wy{eb}")
                nc.sync.dma_start(t[:], wyt[eb])
                wy_sb.append(t)
            enc_sb = cp.tile([128, KD * B_LOC], BF16)
            nc.sync.dma_start(enc_sb[:], enc_cols[:])
            wh_sb = []
            whp_cm = tc.tile_pool(name="wh", bufs=1)
            whp = whp_cm.__enter__()
            for eb in range(EB):
                t = whp.tile([128, KD * 128], BF16, tag=f"wh{eb}")
                nc.sync.dma_start(t[:], wht[eb])
                wh_sb.append(t)
            ones_sb = cp.tile([128, 2], F32R)
            half_sb = cp.tile([128, 1], F32)
            nc.vector.memset(half_sb[:], 0.5)
            wa_sb = cp.tile([128, EB], FP8)
            mask_sb = cp.tile([128, NTOK // 128], F32)
            whhn_sb = cp.tile([128, EB * B_LOC], F32)
            nc.sync.dma_start(ones_sb[:], ones[:])
            nc.sync.dma_start(wa_sb[:], wa_cols[:])
            nc.sync.dma_start(mask_sb[:], mask_cols[:])
            load_xnat(0)

            def emit_whhn():
                for eb in range(EB):
                    php = psum.tile([128, B_LOC], F32, tag="apre", bufs=2,
                                    name=f"php_{eb}")
                    for k in range(KD):
                        nc.tensor.matmul(
                            php[:], wh_sb[eb][:, k * 128:(k + 1) * 128],
                            enc_sb[:, k * B_LOC:(k + 1) * B_LOC],
                            start=(k == 0), stop=(k == KD - 1))
                    nc.vector.tensor_copy(
                        whhn_sb[:, eb * B_LOC:(eb + 1) * B_LOC], php[:])

            # ---- main loop ----
            def emit_z_mm(j, eb):
                zp = psum.tile([128, TILE_T], F32, tag="z", bufs=3,
                               name=f"z_{j}_{eb}")
                xt_t = state[("xt", j)]
                for q in range(KD // 2):
                    nc.tensor.matmul(
                        zp[:],
                        wy_sb[eb][:, q * 256:(q + 1) * 256]
                        .rearrange("p (i m) -> p i m", i=2),
                        xt_t[:, 2 * q * TILE_T:(2 * q + 2) * TILE_T]
                        .rearrange("p (i n) -> p i n", i=2),
                        start=(q == 0), stop=(q == KD // 2 - 1),
                        perf_mode=mybir.MatmulPerfMode.DoubleRow)
                return zp

            def emit_tanh(j, eb, zp):
                b = j // BT_PER_B
                th_t = thp.tile([128, TILE_T], FP8, tag=f"th{eb}",
                                name=f"th_{j}_{eb}")
                nc.scalar.activation(
                    th_t[:], zp[:], AF.Tanh, scale=1.0 / 32.0,
                    bias=whhn_sb[:, eb * B_LOC + b:eb * B_LOC + b + 1])
                state[("th", j, eb)] = th_t

            def emit_z(j, ebs):
                for eb in ebs:
                    emit_tanh(j, eb, emit_z_mm(j, eb))

            def emit_apre(j):
                app = psum.tile([128, CH], F32, tag="apre", bufs=2,
                                name=f"apre_{j}")
                for c in range(CH):
                    for eb in range(EB):
                        nc.tensor.matmul(
                            app[:, c:c + 1],
                            state[("th", j, eb)][:, c * 128:(c + 1) * 128],
                            wa_sb[:, eb:eb + 1],
                            start=(eb == 0), stop=(eb == EB - 1))
                for eb in range(EB):
                    state.pop(("th", j, eb))
                # sigmoid(x) = 0.5 + 0.5*tanh(x/2); both tanh and exp live
                # in the exp_and_others table set -> no table reloads.
                tcol = smp.tile([128, CH], F32, tag="tcol", name=f"tcol_{j}")
                nc.scalar.activation(tcol[:], app[:], AF.Tanh,
                     scale=0.5 / 32.0)
                ex = smp.tile([128, CH], F32, tag="ex", name=f"ex_{j}")
                nc.scalar.activation(ex[:], tcol[:], AF.Exp,
                                     bias=half_sb[:], scale=0.5)
                ew = smp.tile([128, CH], F32R, tag="ew", name=f"ew_{j}")
                nc.vector.tensor_mul(
                    ew[:], ex[:], mask_sb[:, j * CH:(j + 1) * CH])
                state[("ew", j)] = ew

            def emit_pool(j):
                b = j // BT_PER_B
                if j % BT_PER_B == 0:
                    state[("num", b)] = [
                        psum.tile([1, 512], F32, tag=f"num{dn}", bufs=1,
                                  name=f"num_{b}_{dn}")
                        for dn in range(2)]
                    state[("den", b)] = psum.tile([1, 2], F32, tag="den",
                                                  bufs=1, name=f"den_{b}")
                ew = state.pop(("ew", j))
                xn = state.pop(("xn", j))
                first = j % BT_PER_B == 0
                last = j % BT_PER_B == BT_PER_B - 1
                for c in range(CH):
                    st = first and c == 0
                    sp = last and c == CH - 1
                    for dn in range(2):
                        nc.tensor.matmul(
                            state[("num", b)][dn][:],
                            ew[:, c:c + 1],
                            xn[:, c * D + dn * 512:c * D + (dn + 1) * 512],
                            start=st, stop=sp)
                    nc.tensor.matmul(
                        state[("den", b)][:], ew[:, c:c + 1], ones_sb[:],
                        start=st, stop=sp)
                if last:
                    rec = smp.tile([1, 1], F32, tag="rec", name=f"rec_{b}")
                    nc.vector.reciprocal(rec[:], state[("den", b)][:, 0:1])
                    ob = smp.tile([1, D], F32, tag="ob", bufs=2,
                                  name=f"ob_{b}")
                    for dn in range(2):
                        nc.vector.tensor_scalar_mul(
                            ob[:, dn * 512:(dn + 1) * 512],
                            state[("num", b)][dn][:], rec[:])
                    nc.sync.dma_start(out[b:b + 1, :], ob[:])
                    state.pop(("num", b))
                    state.pop(("den", b))

            for j in range(NBT):
                if j + 1 < NBT:
                    load_xt(j + 1)
                    load_xnat(j + 1)
                if j == 0:
                    zps = [emit_z_mm(0, eb) for eb in range(3)]
                    emit_whhn()
                    whp_cm.__exit__(None, None, None)
                    for eb in range(3):
                        emit_tanh(0, eb, zps[eb])
                    emit_z(0, range(3, 4))
                else:
                    emit_z(j, range(0, 2))
                if j > 0:
                    emit_z(j, range(2, 3))
                    emit_apre(j - 1)
                    emit_z(j, range(3, 4))
                if j > 1:
                    emit_z(j, range(4, 5))
                    emit_pool(j - 2)
                    emit_z(j, range(5, 8))
                else:
                    emit_z(j, range(4, 8) if j else range(4, 8))
            emit_apre(NBT - 1)
            emit_pool(NBT - 2)
            emit_pool(NBT - 1)

    nc.compile()
    return nc


def _host_pack(full_input, encoding, mask, W_h, W_y, w_a):
    """Build the per-core input maps (layout transforms / casts only)."""
    wyT = np.ascontiguousarray(W_y.T)  # [d, e]
    whT = np.ascontiguousarray(W_h.T)
    wyt_rows = np.empty((EB, 128, KD * 128), ml_dtypes.float8_e4m3)
    wht_rows = np.empty((EB, 128, KD * 128), ml_dtypes.bfloat16)
    for eb in range(EB):
        for k in range(KD):
            wyt_rows[eb, :, k * 128:(k + 1) * 128] = (
                32.0 * wyT[k * 128:(k + 1) * 128, eb * 128:(eb + 1) * 128])
            wht_rows[eb, :, k * 128:(k + 1) * 128] = \
                whT[k * 128:(k + 1) * 128, eb * 128:(eb + 1) * 128]
    wa_c = np.ascontiguousarray(
        32.0 * w_a.reshape(KD, 128).T).astype(ml_dtypes.float8_e4m3)
    ones = np.ones((128, 2), np.float32)

    in_maps = []
    for i in range(N_CORES):
        sl = slice(i * B_LOC, (i + 1) * B_LOC)
        x_i = np.ascontiguousarray(
            full_input[sl].reshape(NTOK, D).astype(np.float32))
        xt_i = np.ascontiguousarray(x_i.T.astype(ml_dtypes.float8_e4m3))
        enc_i = np.ascontiguousarray(
            encoding[sl].T.reshape(KD, 128, B_LOC).transpose(1, 0, 2)
            .reshape(128, KD * B_LOC)).astype(ml_dtypes.bfloat16)
        mask_i = np.ascontiguousarray(
            mask[sl].reshape(NTOK // 128, 128).T.astype(np.float32))
        in_maps.append({
            "x": x_i, "xt": xt_i, "wyt": wyt_rows, "wht": wht_rows,
            "enc_cols": enc_i, "wa_cols": wa_c, "mask_cols": mask_i,
            "ones": ones,
        })
    return in_maps


def run(inputs, trace=False):
    if "nc" not in _CACHE:
        _CACHE["nc"] = build()
    nc = _CACHE["nc"]
    in_maps = _host_pack(**inputs)
    res = run_bass_kernel_spmd(nc, in_maps, core_ids=list(range(N_CORES)),
                               trace=trace)
    out = np.concatenate([res.results[i]["out"] for i in range(N_CORES)],
                         axis=0)
    return out, res


def kernel(**inputs):
    out, _ = run(inputs, trace=False)
    return out
